# revision 30
# baseline (speedup 1.0000x reference)
"""InfoNCE loss kernel for 8 Trainium2 NeuronCores (symmetric-triangle version).

Math (reference): z = concat(z1, z2) [2N, D] row-normalized; sim = z@z.T/TEMP;
self-diagonal masked; loss = mean(-pos + logsumexp(sim, axis=1)) where
pos[i] = sim[i, partner(i)].

sim is symmetric, so each off-diagonal [512,512] block is computed ONCE and
its exp contributes to row sums (row direction) and column sums (column
direction). The 16x16 grid of 512-blocks has 136 upper-triangle blocks =
8 cores x 17 blocks, assigned by a circulant decomposition: core c owns
blocks (c,c), (c+8,c+8), (c,c+8) [positive pairs], and (c,c+d),
(c+8,c+8+d) for d=1..7 (mod 16). Per-core input panels are rotated by c so
the SPMD graph is identical across cores (slot u holds panel (u+c)%16).

Per core: 272 fp8 DoubleRow matmuls (vs 512 for the full row-block) compute
the 17 blocks; the scalar engine exps two PSUM banks at a time with fp32
row-sum accumulation (accum_out); self-diagonal exp values are extracted and
subtracted on the host instead of masked; column sums are built by a
DVE+GpSimd add chain over the exp tiles and reduced across partitions with a
ones-vector matmul into a [1,512] PSUM row that DMAs straight to DRAM.
The host adds row/col partials across cores, takes ln, and combines with the
positive diagonal (extracted raw from PSUM).

Tricks kept from the row-parallel version: z pre-scaled by 8 on the host
before the fp8e4m3 cast, 1/64 folded into the exp scale; exp computed as
exp(sim/T - 1/T) so sums stay <= 1 per term, host adds the 1/T back.
"""

from contextlib import ExitStack

import ml_dtypes
import numpy as np

import concourse.bass as bass
import concourse.tile as tile
from concourse import bacc, mybir
from concourse.bass_utils import run_bass_kernel_spmd
from concourse.masks import make_identity

N_CORES = 8
N, D = 4096, 1024
ROWS = 2 * N               # 8192 total rows of z
TEMP = 0.07
INV_T = 1.0 / TEMP
FP8_SCALE = 8.0            # host pre-scale before e4m3 cast
MM_SCALE = INV_T / (FP8_SCALE * FP8_SCALE)
NTILE = 512                # rows per z panel / PSUM bank (fp32)
NP = ROWS // NTILE         # 16 panels
KT = D // 128              # 8 contraction slices (4 DoubleRow pairs)

_CACHE = {}

# per-core blocks in slot space (u, v, kind); processed in listed order so
# slot DMA arrival (0,1,2,...,15) stays ahead of consumption.
# kinds: D=self-diagonal, G=generic, P=positive-pair block.
# Grouped into same-u pairs so one [128,2,512] PSUM pair holds the q-th row
# tile of both blocks and a single exp covers both with one fp32 accum_out.
PAIRS = [
    ((0, 8, "P"), None),           # odd block first: its exps+extracts run
    ((0, 0, "D"), (0, 1, "G")),    # during the DMA lead-in instead of
    ((0, 2, "G"), (0, 3, "G")),    # bubbling the scalar stream mid-kernel
    ((0, 4, "G"), (0, 5, "G")),
    ((0, 6, "G"), (0, 7, "G")),
    ((8, 9, "G"), (8, 10, "G")),
    ((8, 11, "G"), (8, 12, "G")),
    ((8, 13, "G"), (8, 14, "G")),
    ((8, 15, "G"), (8, 8, "D")),
]
# DMA issue order follows consumption: pos block (slots 0,8) leads
DMA_ORDER = [0, 8, 1, 2, 3, 4, 5, 6, 7, 9, 10, 11, 12, 13, 14, 15]


def _build_graph():
    nc = bacc.Bacc("TRN2", target_bir_lowering=False, debug=False, num_devices=N_CORES)
    z = nc.declare_dram_parameter("z", [NP, 128, KT, NTILE], mybir.dt.float8e4, isOutput=False)
    out = nc.declare_dram_parameter("out", [128, 20], mybir.dt.float32, isOutput=True)
    colsum = nc.declare_dram_parameter("cols", [15, 128, NTILE], mybir.dt.bfloat16, isOutput=True)
    tailex = nc.declare_dram_parameter("tailex", [4, 128, NTILE], mybir.dt.bfloat16, isOutput=True)

    fp32 = mybir.dt.float32
    bf16 = mybir.dt.bfloat16
    fp8 = mybir.dt.float8e4
    AF = mybir.ActivationFunctionType
    AX = mybir.AxisListType.X
    ALU = mybir.AluOpType
    DR = mybir.MatmulPerfMode.DoubleRow

    with tile.TileContext(nc) as tc, ExitStack() as ctx:
        zpool = ctx.enter_context(tc.tile_pool(name="z", bufs=1))
        consts = ctx.enter_context(tc.tile_pool(name="consts", bufs=1))
        pspool = ctx.enter_context(tc.tile_pool(name="ps", bufs=4, space="PSUM"))
        expool = ctx.enter_context(tc.tile_pool(name="ex", bufs=8))
        afpool = ctx.enter_context(tc.tile_pool(name="af", bufs=3))
        abpool = ctx.enter_context(tc.tile_pool(name="ab", bufs=3))
        outpool = ctx.enter_context(tc.tile_pool(name="outp", bufs=1))

        # stage all 16 z panels into SBUF up front, all on the sync queue:
        # DMA issues gate on prior completions (ring depth ~2), so a queue
        # that later runs compute (scalar/vector) would stall its engine
        # behind the transfer stream. Arrival (~2.5us/panel) stays ahead of
        # consumption (~3.5us/panel).
        zc = [
            zpool.tile([128, KT, NTILE], fp8, tag=f"zc{s}", name=f"zc{s}")
            for s in range(NP)
        ]
        for i, s in enumerate(DMA_ORDER):
            t = zc[s]
            if i < 2:
                # first two panels land in k-halves so the leading matmuls
                # (which touch k 0..3 first) start ~1.5us earlier
                nc.sync.dma_start(out=t[:, 0:4, :], in_=z[s, :, 0:4, :])
                nc.sync.dma_start(out=t[:, 4:8, :], in_=z[s, :, 4:8, :])
            else:
                nc.sync.dma_start(out=t[:], in_=z[s])

        eye = consts.tile([128, 128], fp32, tag="eye")
        make_identity(nc, eye[:])
        nbias = consts.tile([128, 1], fp32, tag="nbias")
        nc.vector.memset(nbias[:], -INV_T)
        # priming activation: forces the Exp ACT_TABLE_LOAD to run during
        # the DMA lead-in instead of right before the first real exp
        prime = consts.tile([128, 1], fp32, tag="prime")
        nc.scalar.activation(
            out=prime[:], in_=nbias[:], func=AF.Exp, bias=nbias[:], scale=1.0
        )
        junk = consts.tile([128, 128], fp32, tag="junk")

        # row-sum partials, cols innermost, summed at the end
        # rowacc0: 4 accum pairs + pos leftover col; rowacc8: 4 accum pairs
        rowacc0 = outpool.tile([128, 4, 5], fp32, tag="rowacc0")
        rowacc8 = outpool.tile([128, 4, 4], fp32, tag="rowacc8")
        # raw self-diagonal sims, extracted from PSUM: (0,0) q0-3, (8,8) q0-3
        draw = outpool.tile([128, 8], fp32, tag="draw")
        # outsb cols: 0:4 rowsum u=0, 4:8 rowsum u=8, 8:16 diag exp values
        # (device-exp of draw, bitwise-matching the accum terms), 16:20 pos
        outsb = outpool.tile([128, 20], fp32, tag="outsb")

        def emit_mms(u, v, q, ps_half):
            """4 DR matmuls: rows 128q..128q+128 of slot u x all 512 of slot v."""
            for k in range(KT // 2):
                nc.tensor.matmul(
                    ps_half,
                    lhsT=zc[u][:, 2 * k : 2 * k + 2, 128 * q : 128 * q + 128],
                    rhs=zc[v][:, 2 * k : 2 * k + 2, :],
                    start=(k == 0),
                    stop=(k == KT // 2 - 1),
                    perf_mode=DR,
                )

        def ship_colacc(ab, v):
            # per-block column accumulator -> DRAM; host reduces partitions.
            # gpsimd queue: follows the add chain in-queue, so it never makes
            # another engine block on the chain; sync ring stays free for the
            # z panel stream and scalar stays free for exps.
            nc.gpsimd.dma_start(out=colsum[v - 1], in_=ab[:])

        npair0 = 0  # accum col index within rowacc0
        npair8 = 0
        for ip, (b1, b2) in enumerate(PAIRS):
            last_pair = ip == len(PAIRS) - 1
            u = b1[0]
            rowacc = rowacc0 if u == 0 else rowacc8
            if b2 is not None:
                # --- standard pair: psum halves = (b1 q-tile, b2 q-tile) ---
                hD = 0 if b1[2] == "D" else (1 if b2[2] == "D" else None)
                pidx = npair0 if u == 0 else npair8
                if u == 0:
                    npair0 += 1
                else:
                    npair8 += 1
                exs = []
                for q in range(4):
                    ps = pspool.tile([128, 2, NTILE], fp32, tag="ps", name="ps")
                    emit_mms(u, b1[1], q, ps[:, 0, :])
                    emit_mms(u, b2[1], q, ps[:, 1, :])
                    if hD is not None:
                        # self-diagonal: row-accum includes it in fp32; pull
                        # the raw sim off PSUM, re-exp it on-device (same
                        # table, same input -> identical value), host
                        # subtracts it exactly
                        dq = 0 if u == 0 else 4
                        nc.vector.tensor_mul(
                            junk[:], ps[:, hD, 128 * q : 128 * q + 128], eye[:]
                        )
                        nc.vector.reduce_sum(
                            draw[:, dq + q : dq + q + 1], junk[:], axis=AX
                        )
                    ex = expool.tile([128, 2, NTILE], bf16, tag="ex")
                    nc.scalar.activation(
                        out=ex[:], in_=ps[:], func=AF.Exp, bias=nbias[:],
                        scale=MM_SCALE, accum_out=rowacc[:, q, pidx : pidx + 1],
                    )
                    exs.append(ex)
                for h, blk in ((0, b1), (1, b2)):
                    if blk[2] == "D":
                        pass  # row-accum handled it; no column sums
                    elif last_pair:
                        # tail block: ship exp halves as they land instead of
                        # running the add chain after the final exp
                        for q in range(4):
                            nc.sync.dma_start(out=tailex[q], in_=exs[q][:, h, :])
                    else:
                        af = afpool.tile([128, NTILE], fp32, tag="af")
                        nc.vector.tensor_add(af[:], exs[0][:, h, :], exs[1][:, h, :])
                        af2 = afpool.tile([128, NTILE], fp32, tag="af")
                        nc.vector.tensor_add(af2[:], af[:], exs[2][:, h, :])
                        ab = abpool.tile([128, NTILE], bf16, tag="ab")
                        nc.vector.tensor_add(ab[:], af2[:], exs[3][:, h, :])
                        ship_colacc(ab, blk[1])
            else:
                # --- leftover positive block: psum halves = (q, q+1) of b1 ---
                v = b1[1]
                for p2 in range(2):
                    ps = pspool.tile([128, 2, NTILE], fp32, tag="ps", name="ps")
                    emit_mms(u, v, 2 * p2, ps[:, 0, :])
                    emit_mms(u, v, 2 * p2 + 1, ps[:, 1, :])
                    # raw positive dots from PSUM before exp
                    for h in range(2):
                        q = 2 * p2 + h
                        nc.vector.tensor_mul(
                            junk[:], ps[:, h, 128 * q : 128 * q + 128], eye[:]
                        )
                        nc.vector.reduce_sum(
                            outsb[:, 16 + q : 16 + q + 1], junk[:], axis=AX
                        )
                    ex = expool.tile([128, 2, NTILE], bf16, tag="ex")
                    # per-half exps: halves are different row groups, so each
                    # gets its own fp32 row accum (no DVE reduce needed)
                    for h in range(2):
                        nc.scalar.activation(
                            out=ex[:, h, :], in_=ps[:, h, :], func=AF.Exp,
                            bias=nbias[:], scale=MM_SCALE,
                            accum_out=rowacc0[:, 2 * p2 + h, 4:5],
                        )
                    if p2 == 0:
                        af = afpool.tile([128, NTILE], fp32, tag="af")
                        nc.vector.tensor_add(af[:], ex[:, 0, :], ex[:, 1, :])
                    else:
                        af2 = afpool.tile([128, NTILE], fp32, tag="af")
                        nc.vector.tensor_add(af2[:], af[:], ex[:, 0, :])
                        ab = abpool.tile([128, NTILE], bf16, tag="ab")
                        nc.vector.tensor_add(ab[:], af2[:], ex[:, 1, :])
                        ship_colacc(ab, v)

        # fold per-pair row accums, exp the raw diagonals, ship everything
        nc.vector.reduce_sum(outsb[:, 0:4], rowacc0[:], axis=AX)
        nc.vector.reduce_sum(outsb[:, 4:8], rowacc8[:], axis=AX)
        nc.scalar.activation(
            out=outsb[:, 8:16], in_=draw[:], func=AF.Exp, bias=nbias[:],
            scale=MM_SCALE,
        )
        nc.scalar.dma_start(out=out[:], in_=outsb[:])

    nc.compile()
    return nc


def _make_in_maps(z1: np.ndarray, z2: np.ndarray):
    z = np.concatenate([z1, z2], axis=0)  # [8192, 1024] f32
    zt = (z * FP8_SCALE).astype(np.float32).T  # [D, ROWS]
    # [NP, 128, KT, NTILE]: per 512-row panel, contiguous [kp, k, n] tiles
    zcb = np.ascontiguousarray(
        zt.reshape(KT, 128, NP, NTILE).transpose(2, 1, 0, 3)
    ).astype(ml_dtypes.float8_e4m3)
    in_maps = []
    for c in range(N_CORES):
        order = [(s + c) % NP for s in range(NP)]
        in_maps.append({"z": np.ascontiguousarray(zcb[order])})
    return in_maps


def kernel(z1: np.ndarray, z2: np.ndarray) -> np.ndarray:
    assert z1.shape == (N, D) and z2.shape == (N, D)
    in_maps = _make_in_maps(z1, z2)

    if "nc" not in _CACHE:
        _CACHE["nc"] = _build_graph()
    res = run_bass_kernel_spmd(_CACHE["nc"], in_maps, core_ids=list(range(N_CORES)))

    S = np.zeros((NP, NTILE), dtype=np.float64)
    pos_sum = 0.0
    for c, r in enumerate(res.results):
        o = np.asarray(r["out"], dtype=np.float64)      # [128, 20]
        cols = np.asarray(r["cols"], dtype=np.float64)  # [15, 128, 512]
        p0, p8 = c, (8 + c) % NP
        for q in range(4):
            sl = slice(128 * q, 128 * q + 128)
            S[p0, sl] += o[:, q] - o[:, 8 + q]
            S[p8, sl] += o[:, 4 + q] - o[:, 12 + q]
        colr = cols.sum(axis=1)
        colr[14] = np.asarray(r["tailex"], dtype=np.float64).sum(axis=(0, 1))                         # [15, 512]
        for v in range(1, NP):
            S[(v + c) % NP] += colr[v - 1]
        pos_sum += o[:, 16:20].sum() * MM_SCALE
    loss = np.log(S).mean() + INV_T - pos_sum / N
    return np.asarray(loss, dtype=np.float32)


# revision 31
# speedup vs baseline: 1.0573x; 1.0573x over previous
"""InfoNCE loss kernel for 8 Trainium2 NeuronCores (symmetric-triangle version).

Math (reference): z = concat(z1, z2) [2N, D] row-normalized; sim = z@z.T/TEMP;
self-diagonal masked; loss = mean(-pos + logsumexp(sim, axis=1)) where
pos[i] = sim[i, partner(i)].

sim is symmetric, so each off-diagonal [512,512] block is computed ONCE and
its exp contributes to row sums (row direction) and column sums (column
direction). The 16x16 grid of 512-blocks has 136 upper-triangle blocks =
8 cores x 17 blocks, assigned by a circulant decomposition: core c owns
blocks (c,c), (c+8,c+8), (c,c+8) [positive pairs], and (c,c+d),
(c+8,c+8+d) for d=1..7 (mod 16). Per-core input panels are rotated by c so
the SPMD graph is identical across cores (slot u holds panel (u+c)%16).

Per core: 272 fp8 DoubleRow matmuls (vs 512 for the full row-block) compute
the 17 blocks; the scalar engine exps two PSUM banks at a time with fp32
row-sum accumulation (accum_out); self-diagonal exp values are extracted and
subtracted on the host instead of masked; column sums are built by a
DVE+GpSimd add chain over the exp tiles and reduced across partitions with a
ones-vector matmul into a [1,512] PSUM row that DMAs straight to DRAM.
The host adds row/col partials across cores, takes ln, and combines with the
positive diagonal (extracted raw from PSUM).

Tricks kept from the row-parallel version: z pre-scaled by 8 on the host
before the fp8e4m3 cast, 1/64 folded into the exp scale; exp computed as
exp(sim/T - 1/T) so sums stay <= 1 per term, host adds the 1/T back.
"""

from contextlib import ExitStack

import ml_dtypes
import numpy as np

import concourse.bass as bass
import concourse.tile as tile
from concourse import bacc, mybir
from concourse.bass_utils import run_bass_kernel_spmd
from concourse.masks import make_identity

N_CORES = 8
N, D = 4096, 1024
ROWS = 2 * N               # 8192 total rows of z
TEMP = 0.07
INV_T = 1.0 / TEMP
FP8_SCALE = 8.0            # host pre-scale before e4m3 cast
MM_SCALE = INV_T / (FP8_SCALE * FP8_SCALE)
NTILE = 512                # rows per z panel / PSUM bank (fp32)
NP = ROWS // NTILE         # 16 panels
KT = D // 128              # 8 contraction slices (4 DoubleRow pairs)

_CACHE = {}

# per-core blocks in slot space (u, v, kind); processed in listed order so
# slot DMA arrival (0,1,2,...,15) stays ahead of consumption.
# kinds: D=self-diagonal, G=generic, P=positive-pair block.
# Grouped into same-u pairs so one [128,2,512] PSUM pair holds the q-th row
# tile of both blocks and a single exp covers both with one fp32 accum_out.
PAIRS = [
    ((0, 0, "D"), (0, 1, "G")),    # pair A leads: only slots 0,1 needed
    ((0, 8, "P"), None),           # odd block early: its scalar surplus
    ((0, 2, "G"), (0, 3, "G")),    # lands in lead-in slack instead of
    ((0, 4, "G"), (0, 5, "G")),    # bubbling the exp stream mid-kernel
    ((0, 6, "G"), (0, 7, "G")),
    ((8, 9, "G"), (8, 10, "G")),
    ((8, 11, "G"), (8, 12, "G")),
    ((8, 13, "G"), (8, 14, "G")),
    ((8, 15, "G"), (8, 8, "D")),
]
# DMA issue order follows consumption
DMA_ORDER = [0, 1, 8, 2, 3, 4, 5, 6, 7, 9, 10, 11, 12, 13, 14, 15]


def _build_graph():
    nc = bacc.Bacc("TRN2", target_bir_lowering=False, debug=False, num_devices=N_CORES)
    z = nc.declare_dram_parameter("z", [NP, 128, KT, NTILE], mybir.dt.float8e4, isOutput=False)
    out = nc.declare_dram_parameter("out", [128, 20], mybir.dt.float32, isOutput=True)
    colsum = nc.declare_dram_parameter("cols", [15, 128, NTILE], mybir.dt.bfloat16, isOutput=True)
    tailex = nc.declare_dram_parameter("tailex", [4, 128, NTILE], mybir.dt.bfloat16, isOutput=True)

    fp32 = mybir.dt.float32
    bf16 = mybir.dt.bfloat16
    fp8 = mybir.dt.float8e4
    AF = mybir.ActivationFunctionType
    AX = mybir.AxisListType.X
    ALU = mybir.AluOpType
    DR = mybir.MatmulPerfMode.DoubleRow

    with tile.TileContext(nc) as tc, ExitStack() as ctx:
        zpool = ctx.enter_context(tc.tile_pool(name="z", bufs=1))
        consts = ctx.enter_context(tc.tile_pool(name="consts", bufs=1))
        pspool = ctx.enter_context(tc.tile_pool(name="ps", bufs=4, space="PSUM"))
        expool = ctx.enter_context(tc.tile_pool(name="ex", bufs=8))
        afpool = ctx.enter_context(tc.tile_pool(name="af", bufs=3))
        abpool = ctx.enter_context(tc.tile_pool(name="ab", bufs=3))
        outpool = ctx.enter_context(tc.tile_pool(name="outp", bufs=1))

        # stage all 16 z panels into SBUF up front, all on the sync queue:
        # DMA issues gate on prior completions (ring depth ~2), so a queue
        # that later runs compute (scalar/vector) would stall its engine
        # behind the transfer stream. Arrival (~2.5us/panel) stays ahead of
        # consumption (~3.5us/panel).
        zc = [
            zpool.tile([128, KT, NTILE], fp8, tag=f"zc{s}", name=f"zc{s}")
            for s in range(NP)
        ]
        for i, s in enumerate(DMA_ORDER):
            t = zc[s]
            if i < 2:
                # first two panels land in k-halves so the leading matmuls
                # (which touch k 0..3 first) start ~1.5us earlier
                nc.sync.dma_start(out=t[:, 0:4, :], in_=z[s, :, 0:4, :])
                nc.sync.dma_start(out=t[:, 4:8, :], in_=z[s, :, 4:8, :])
            else:
                nc.sync.dma_start(out=t[:], in_=z[s])

        eye = consts.tile([128, 128], fp32, tag="eye")
        make_identity(nc, eye[:])
        nbias = consts.tile([128, 1], fp32, tag="nbias")
        nc.vector.memset(nbias[:], -INV_T)
        # priming activation: forces the Exp ACT_TABLE_LOAD to run during
        # the DMA lead-in instead of right before the first real exp
        prime = consts.tile([128, 1], fp32, tag="prime")
        nc.scalar.activation(
            out=prime[:], in_=nbias[:], func=AF.Exp, bias=nbias[:], scale=1.0
        )
        junk = consts.tile([128, 128], fp32, tag="junk")

        # row-sum partials, cols innermost, summed at the end
        # rowacc0: 4 accum pairs + pos leftover col; rowacc8: 4 accum pairs
        rowacc0 = outpool.tile([128, 4, 5], fp32, tag="rowacc0")
        rowacc8 = outpool.tile([128, 4, 4], fp32, tag="rowacc8")
        # raw self-diagonal sims, extracted from PSUM: (0,0) q0-3, (8,8) q0-3
        draw = outpool.tile([128, 8], fp32, tag="draw")
        # outsb cols: 0:4 rowsum u=0, 4:8 rowsum u=8, 8:16 diag exp values
        # (device-exp of draw, bitwise-matching the accum terms), 16:20 pos
        outsb = outpool.tile([128, 20], fp32, tag="outsb")

        def emit_mms(u, v, q, ps_half):
            """4 DR matmuls: rows 128q..128q+128 of slot u x all 512 of slot v."""
            for k in range(KT // 2):
                nc.tensor.matmul(
                    ps_half,
                    lhsT=zc[u][:, 2 * k : 2 * k + 2, 128 * q : 128 * q + 128],
                    rhs=zc[v][:, 2 * k : 2 * k + 2, :],
                    start=(k == 0),
                    stop=(k == KT // 2 - 1),
                    perf_mode=DR,
                )

        def ship_colacc(ab, v):
            # per-block column accumulator -> DRAM; host reduces partitions.
            # gpsimd queue: follows the add chain in-queue, so it never makes
            # another engine block on the chain; sync ring stays free for the
            # z panel stream and scalar stays free for exps.
            nc.gpsimd.dma_start(out=colsum[v - 1], in_=ab[:])

        npair0 = 0  # accum col index within rowacc0
        npair8 = 0
        for ip, (b1, b2) in enumerate(PAIRS):
            last_pair = ip == len(PAIRS) - 1
            u = b1[0]
            rowacc = rowacc0 if u == 0 else rowacc8
            if b2 is not None:
                # --- standard pair: psum halves = (b1 q-tile, b2 q-tile) ---
                hD = 0 if b1[2] == "D" else (1 if b2[2] == "D" else None)
                pidx = npair0 if u == 0 else npair8
                if u == 0:
                    npair0 += 1
                else:
                    npair8 += 1
                exs = []
                for q in range(4):
                    ps = pspool.tile([128, 2, NTILE], fp32, tag="ps", name="ps")
                    emit_mms(u, b1[1], q, ps[:, 0, :])
                    emit_mms(u, b2[1], q, ps[:, 1, :])
                    if hD is not None:
                        # self-diagonal: row-accum includes it in fp32; pull
                        # the raw sim off PSUM, re-exp it on-device (same
                        # table, same input -> identical value), host
                        # subtracts it exactly
                        dq = 0 if u == 0 else 4
                        nc.vector.tensor_mul(
                            junk[:], ps[:, hD, 128 * q : 128 * q + 128], eye[:]
                        )
                        nc.vector.reduce_sum(
                            draw[:, dq + q : dq + q + 1], junk[:], axis=AX
                        )
                    ex = expool.tile([128, 2, NTILE], bf16, tag="ex")
                    nc.scalar.activation(
                        out=ex[:], in_=ps[:], func=AF.Exp, bias=nbias[:],
                        scale=MM_SCALE, accum_out=rowacc[:, q, pidx : pidx + 1],
                    )
                    exs.append(ex)
                for h, blk in ((0, b1), (1, b2)):
                    if blk[2] == "D":
                        pass  # row-accum handled it; no column sums
                    elif last_pair:
                        # tail block: ship exp halves as they land instead of
                        # running the add chain after the final exp
                        for q in range(4):
                            nc.sync.dma_start(out=tailex[q], in_=exs[q][:, h, :])
                    else:
                        af = afpool.tile([128, NTILE], fp32, tag="af")
                        nc.vector.tensor_add(af[:], exs[0][:, h, :], exs[1][:, h, :])
                        af2 = afpool.tile([128, NTILE], fp32, tag="af")
                        nc.vector.tensor_add(af2[:], af[:], exs[2][:, h, :])
                        ab = abpool.tile([128, NTILE], bf16, tag="ab")
                        nc.vector.tensor_add(ab[:], af2[:], exs[3][:, h, :])
                        ship_colacc(ab, blk[1])
            else:
                # --- leftover positive block: psum halves = (q, q+1) of b1 ---
                v = b1[1]
                for p2 in range(2):
                    ps = pspool.tile([128, 2, NTILE], fp32, tag="ps", name="ps")
                    emit_mms(u, v, 2 * p2, ps[:, 0, :])
                    emit_mms(u, v, 2 * p2 + 1, ps[:, 1, :])
                    # raw positive dots from PSUM before exp
                    for h in range(2):
                        q = 2 * p2 + h
                        nc.vector.tensor_mul(
                            junk[:], ps[:, h, 128 * q : 128 * q + 128], eye[:]
                        )
                        nc.vector.reduce_sum(
                            outsb[:, 16 + q : 16 + q + 1], junk[:], axis=AX
                        )
                    ex = expool.tile([128, 2, NTILE], bf16, tag="ex")
                    # per-half exps: halves are different row groups, so each
                    # gets its own fp32 row accum (no DVE reduce needed)
                    for h in range(2):
                        nc.scalar.activation(
                            out=ex[:, h, :], in_=ps[:, h, :], func=AF.Exp,
                            bias=nbias[:], scale=MM_SCALE,
                            accum_out=rowacc0[:, 2 * p2 + h, 4:5],
                        )
                    if p2 == 0:
                        af = afpool.tile([128, NTILE], fp32, tag="af")
                        nc.vector.tensor_add(af[:], ex[:, 0, :], ex[:, 1, :])
                    else:
                        af2 = afpool.tile([128, NTILE], fp32, tag="af")
                        nc.vector.tensor_add(af2[:], af[:], ex[:, 0, :])
                        ab = abpool.tile([128, NTILE], bf16, tag="ab")
                        nc.vector.tensor_add(ab[:], af2[:], ex[:, 1, :])
                        ship_colacc(ab, v)

        # fold per-pair row accums, exp the raw diagonals, ship everything
        nc.vector.reduce_sum(outsb[:, 0:4], rowacc0[:], axis=AX)
        nc.vector.reduce_sum(outsb[:, 4:8], rowacc8[:], axis=AX)
        nc.scalar.activation(
            out=outsb[:, 8:16], in_=draw[:], func=AF.Exp, bias=nbias[:],
            scale=MM_SCALE,
        )
        nc.scalar.dma_start(out=out[:], in_=outsb[:])

    nc.compile()
    return nc


def _make_in_maps(z1: np.ndarray, z2: np.ndarray):
    z = np.concatenate([z1, z2], axis=0)  # [8192, 1024] f32
    zt = (z * FP8_SCALE).astype(np.float32).T  # [D, ROWS]
    # [NP, 128, KT, NTILE]: per 512-row panel, contiguous [kp, k, n] tiles
    zcb = np.ascontiguousarray(
        zt.reshape(KT, 128, NP, NTILE).transpose(2, 1, 0, 3)
    ).astype(ml_dtypes.float8_e4m3)
    in_maps = []
    for c in range(N_CORES):
        order = [(s + c) % NP for s in range(NP)]
        in_maps.append({"z": np.ascontiguousarray(zcb[order])})
    return in_maps


def kernel(z1: np.ndarray, z2: np.ndarray) -> np.ndarray:
    assert z1.shape == (N, D) and z2.shape == (N, D)
    in_maps = _make_in_maps(z1, z2)

    if "nc" not in _CACHE:
        _CACHE["nc"] = _build_graph()
    res = run_bass_kernel_spmd(_CACHE["nc"], in_maps, core_ids=list(range(N_CORES)))

    S = np.zeros((NP, NTILE), dtype=np.float64)
    pos_sum = 0.0
    for c, r in enumerate(res.results):
        o = np.asarray(r["out"], dtype=np.float64)      # [128, 20]
        cols = np.asarray(r["cols"], dtype=np.float64)  # [15, 128, 512]
        p0, p8 = c, (8 + c) % NP
        for q in range(4):
            sl = slice(128 * q, 128 * q + 128)
            S[p0, sl] += o[:, q] - o[:, 8 + q]
            S[p8, sl] += o[:, 4 + q] - o[:, 12 + q]
        colr = cols.sum(axis=1)
        colr[14] = np.asarray(r["tailex"], dtype=np.float64).sum(axis=(0, 1))                         # [15, 512]
        for v in range(1, NP):
            S[(v + c) % NP] += colr[v - 1]
        pos_sum += o[:, 16:20].sum() * MM_SCALE
    loss = np.log(S).mean() + INV_T - pos_sum / N
    return np.asarray(loss, dtype=np.float32)


# revision 32
# speedup vs baseline: 1.0732x; 1.0150x over previous
"""InfoNCE loss kernel for 8 Trainium2 NeuronCores (symmetric-triangle version).

Math (reference): z = concat(z1, z2) [2N, D] row-normalized; sim = z@z.T/TEMP;
self-diagonal masked; loss = mean(-pos + logsumexp(sim, axis=1)) where
pos[i] = sim[i, partner(i)].

sim is symmetric, so each off-diagonal [512,512] block is computed ONCE and
its exp contributes to row sums (row direction) and column sums (column
direction). The 16x16 grid of 512-blocks has 136 upper-triangle blocks =
8 cores x 17 blocks, assigned by a circulant decomposition: core c owns
blocks (c,c), (c+8,c+8), (c,c+8) [positive pairs], and (c,c+d),
(c+8,c+8+d) for d=1..7 (mod 16). Per-core input panels are rotated by c so
the SPMD graph is identical across cores (slot u holds panel (u+c)%16).

Per core: 272 fp8 DoubleRow matmuls (vs 512 for the full row-block) compute
the 17 blocks; the scalar engine exps two PSUM banks at a time with fp32
row-sum accumulation (accum_out); self-diagonal exp values are extracted and
subtracted on the host instead of masked; column sums are built by a
DVE+GpSimd add chain over the exp tiles and reduced across partitions with a
ones-vector matmul into a [1,512] PSUM row that DMAs straight to DRAM.
The host adds row/col partials across cores, takes ln, and combines with the
positive diagonal (extracted raw from PSUM).

Tricks kept from the row-parallel version: z pre-scaled by 8 on the host
before the fp8e4m3 cast, 1/64 folded into the exp scale; exp computed as
exp(sim/T - 1/T) so sums stay <= 1 per term, host adds the 1/T back.
"""

from contextlib import ExitStack

import ml_dtypes
import numpy as np

import concourse.bass as bass
import concourse.tile as tile
from concourse import bacc, mybir
from concourse.bass_utils import run_bass_kernel_spmd
from concourse.masks import make_identity

N_CORES = 8
N, D = 4096, 1024
ROWS = 2 * N               # 8192 total rows of z
TEMP = 0.07
INV_T = 1.0 / TEMP
FP8_SCALE = 8.0            # host pre-scale before e4m3 cast
MM_SCALE = INV_T / (FP8_SCALE * FP8_SCALE)
NTILE = 512                # rows per z panel / PSUM bank (fp32)
NP = ROWS // NTILE         # 16 panels
KT = D // 128              # 8 contraction slices (4 DoubleRow pairs)

_CACHE = {}

# per-core blocks in slot space (u, v, kind); processed in listed order so
# slot DMA arrival (0,1,2,...,15) stays ahead of consumption.
# kinds: D=self-diagonal, G=generic, P=positive-pair block.
# Grouped into same-u pairs so one [128,2,512] PSUM pair holds the q-th row
# tile of both blocks and a single exp covers both with one fp32 accum_out.
PAIRS = [
    ((0, 0, "D"), (0, 1, "G")),    # pair A leads: only slots 0,1 needed
    ((0, 8, "P"), None),           # odd block early: its scalar surplus
    ((0, 2, "G"), (0, 3, "G")),    # lands in lead-in slack instead of
    ((0, 4, "G"), (0, 5, "G")),    # bubbling the exp stream mid-kernel
    ((0, 6, "G"), (0, 7, "G")),
    ((8, 9, "G"), (8, 10, "G")),
    ((8, 11, "G"), (8, 12, "G")),
    ((8, 13, "G"), (8, 14, "G")),
    ((8, 15, "G"), (8, 8, "D")),
]
# DMA issue order follows consumption
DMA_ORDER = [0, 1, 8, 2, 3, 4, 5, 6, 7, 9, 10, 11, 12, 13, 14, 15]


def _build_graph():
    nc = bacc.Bacc("TRN2", target_bir_lowering=False, debug=False, num_devices=N_CORES)
    z = nc.declare_dram_parameter("z", [NP, 128, KT, NTILE], mybir.dt.float8e4, isOutput=False)
    out = nc.declare_dram_parameter("out", [128, 20], mybir.dt.float32, isOutput=True)
    colsum = nc.declare_dram_parameter("cols", [15, 128, NTILE], mybir.dt.bfloat16, isOutput=True)
    tailex = nc.declare_dram_parameter("tailex", [4, 128, NTILE], mybir.dt.bfloat16, isOutput=True)

    fp32 = mybir.dt.float32
    bf16 = mybir.dt.bfloat16
    fp8 = mybir.dt.float8e4
    AF = mybir.ActivationFunctionType
    AX = mybir.AxisListType.X
    ALU = mybir.AluOpType
    DR = mybir.MatmulPerfMode.DoubleRow

    with tile.TileContext(nc) as tc, ExitStack() as ctx:
        zpool = ctx.enter_context(tc.tile_pool(name="z", bufs=1))
        consts = ctx.enter_context(tc.tile_pool(name="consts", bufs=1))
        pspool = ctx.enter_context(tc.tile_pool(name="ps", bufs=4, space="PSUM"))
        expool = ctx.enter_context(tc.tile_pool(name="ex", bufs=8))
        afpool = ctx.enter_context(tc.tile_pool(name="af", bufs=3))
        abpool = ctx.enter_context(tc.tile_pool(name="ab", bufs=3))
        outpool = ctx.enter_context(tc.tile_pool(name="outp", bufs=1))

        # stage all 16 z panels into SBUF up front, all on the sync queue:
        # DMA issues gate on prior completions (ring depth ~2), so a queue
        # that later runs compute (scalar/vector) would stall its engine
        # behind the transfer stream. Arrival (~2.5us/panel) stays ahead of
        # consumption (~3.5us/panel).
        zc = [
            zpool.tile([128, KT, NTILE], fp8, tag=f"zc{s}", name=f"zc{s}")
            for s in range(NP)
        ]
        for i, s in enumerate(DMA_ORDER):
            t = zc[s]
            if i < 2:
                # first two panels land in k-halves so the leading matmuls
                # (which touch k 0..3 first) start ~1.5us earlier
                nc.sync.dma_start(out=t[:, 0:4, :], in_=z[s, :, 0:4, :])
                nc.sync.dma_start(out=t[:, 4:8, :], in_=z[s, :, 4:8, :])
            else:
                nc.sync.dma_start(out=t[:], in_=z[s])

        eye = consts.tile([128, 128], fp32, tag="eye")
        make_identity(nc, eye[:])
        # PE warm-up: a few throwaway matmuls during the DMA lead-in climb
        # the p-state ladder so the real stream starts at full clock
        wsrc = consts.tile([128, 2, NTILE], fp8, tag="wsrc")
        nc.gpsimd.memset(wsrc[:], 0.0)
        wps = pspool.tile([128, 2, NTILE], fp32, tag="ps", name="ps")
        for w in range(6):
            nc.tensor.matmul(
                wps[:, 0, :], lhsT=wsrc[:, :, 0:128], rhs=wsrc[:],
                start=(w == 0), stop=(w == 5), perf_mode=DR,
            )
        nbias = consts.tile([128, 1], fp32, tag="nbias")
        nc.vector.memset(nbias[:], -INV_T)
        # priming activation: forces the Exp ACT_TABLE_LOAD to run during
        # the DMA lead-in instead of right before the first real exp
        prime = consts.tile([128, 1], fp32, tag="prime")
        nc.scalar.activation(
            out=prime[:], in_=nbias[:], func=AF.Exp, bias=nbias[:], scale=1.0
        )
        junk = consts.tile([128, 128], fp32, tag="junk")

        # row-sum partials, cols innermost, summed at the end
        # rowacc0: 4 accum pairs + pos leftover col; rowacc8: 4 accum pairs
        rowacc0 = outpool.tile([128, 4, 5], fp32, tag="rowacc0")
        rowacc8 = outpool.tile([128, 4, 4], fp32, tag="rowacc8")
        # raw self-diagonal sims, extracted from PSUM: (0,0) q0-3, (8,8) q0-3
        draw = outpool.tile([128, 8], fp32, tag="draw")
        # outsb cols: 0:4 rowsum u=0, 4:8 rowsum u=8, 8:16 diag exp values
        # (device-exp of draw, bitwise-matching the accum terms), 16:20 pos
        outsb = outpool.tile([128, 20], fp32, tag="outsb")

        def emit_mms(u, v, q, ps_half):
            """4 DR matmuls: rows 128q..128q+128 of slot u x all 512 of slot v."""
            for k in range(KT // 2):
                nc.tensor.matmul(
                    ps_half,
                    lhsT=zc[u][:, 2 * k : 2 * k + 2, 128 * q : 128 * q + 128],
                    rhs=zc[v][:, 2 * k : 2 * k + 2, :],
                    start=(k == 0),
                    stop=(k == KT // 2 - 1),
                    perf_mode=DR,
                )

        def ship_colacc(ab, v):
            # per-block column accumulator -> DRAM; host reduces partitions.
            # gpsimd queue: follows the add chain in-queue, so it never makes
            # another engine block on the chain; sync ring stays free for the
            # z panel stream and scalar stays free for exps.
            nc.gpsimd.dma_start(out=colsum[v - 1], in_=ab[:])

        npair0 = 0  # accum col index within rowacc0
        npair8 = 0
        for ip, (b1, b2) in enumerate(PAIRS):
            last_pair = ip == len(PAIRS) - 1
            u = b1[0]
            rowacc = rowacc0 if u == 0 else rowacc8
            if b2 is not None:
                # --- standard pair: psum halves = (b1 q-tile, b2 q-tile) ---
                hD = 0 if b1[2] == "D" else (1 if b2[2] == "D" else None)
                pidx = npair0 if u == 0 else npair8
                if u == 0:
                    npair0 += 1
                else:
                    npair8 += 1
                exs = []
                for q in range(4):
                    ps = pspool.tile([128, 2, NTILE], fp32, tag="ps", name="ps")
                    emit_mms(u, b1[1], q, ps[:, 0, :])
                    emit_mms(u, b2[1], q, ps[:, 1, :])
                    if hD is not None:
                        # self-diagonal: row-accum includes it in fp32; pull
                        # the raw sim off PSUM, re-exp it on-device (same
                        # table, same input -> identical value), host
                        # subtracts it exactly
                        dq = 0 if u == 0 else 4
                        nc.vector.tensor_mul(
                            junk[:], ps[:, hD, 128 * q : 128 * q + 128], eye[:]
                        )
                        nc.vector.reduce_sum(
                            draw[:, dq + q : dq + q + 1], junk[:], axis=AX
                        )
                    ex = expool.tile([128, 2, NTILE], bf16, tag="ex")
                    nc.scalar.activation(
                        out=ex[:], in_=ps[:], func=AF.Exp, bias=nbias[:],
                        scale=MM_SCALE, accum_out=rowacc[:, q, pidx : pidx + 1],
                    )
                    exs.append(ex)
                for h, blk in ((0, b1), (1, b2)):
                    if blk[2] == "D":
                        pass  # row-accum handled it; no column sums
                    elif last_pair:
                        # tail block: ship exp halves as they land instead of
                        # running the add chain after the final exp
                        for q in range(4):
                            nc.sync.dma_start(out=tailex[q], in_=exs[q][:, h, :])
                    else:
                        af = afpool.tile([128, NTILE], fp32, tag="af")
                        nc.vector.tensor_add(af[:], exs[0][:, h, :], exs[1][:, h, :])
                        af2 = afpool.tile([128, NTILE], fp32, tag="af")
                        nc.vector.tensor_add(af2[:], af[:], exs[2][:, h, :])
                        ab = abpool.tile([128, NTILE], bf16, tag="ab")
                        nc.vector.tensor_add(ab[:], af2[:], exs[3][:, h, :])
                        ship_colacc(ab, blk[1])
            else:
                # --- leftover positive block: psum halves = (q, q+1) of b1 ---
                v = b1[1]
                for p2 in range(2):
                    ps = pspool.tile([128, 2, NTILE], fp32, tag="ps", name="ps")
                    emit_mms(u, v, 2 * p2, ps[:, 0, :])
                    emit_mms(u, v, 2 * p2 + 1, ps[:, 1, :])
                    # raw positive dots from PSUM before exp
                    for h in range(2):
                        q = 2 * p2 + h
                        nc.vector.tensor_mul(
                            junk[:], ps[:, h, 128 * q : 128 * q + 128], eye[:]
                        )
                        nc.vector.reduce_sum(
                            outsb[:, 16 + q : 16 + q + 1], junk[:], axis=AX
                        )
                    ex = expool.tile([128, 2, NTILE], bf16, tag="ex")
                    # per-half exps: halves are different row groups, so each
                    # gets its own fp32 row accum (no DVE reduce needed)
                    for h in range(2):
                        nc.scalar.activation(
                            out=ex[:, h, :], in_=ps[:, h, :], func=AF.Exp,
                            bias=nbias[:], scale=MM_SCALE,
                            accum_out=rowacc0[:, 2 * p2 + h, 4:5],
                        )
                    if p2 == 0:
                        af = afpool.tile([128, NTILE], fp32, tag="af")
                        nc.vector.tensor_add(af[:], ex[:, 0, :], ex[:, 1, :])
                    else:
                        af2 = afpool.tile([128, NTILE], fp32, tag="af")
                        nc.vector.tensor_add(af2[:], af[:], ex[:, 0, :])
                        ab = abpool.tile([128, NTILE], bf16, tag="ab")
                        nc.vector.tensor_add(ab[:], af2[:], ex[:, 1, :])
                        ship_colacc(ab, v)

        # fold per-pair row accums, exp the raw diagonals, ship everything
        nc.vector.reduce_sum(outsb[:, 0:4], rowacc0[:], axis=AX)
        nc.vector.reduce_sum(outsb[:, 4:8], rowacc8[:], axis=AX)
        nc.scalar.activation(
            out=outsb[:, 8:16], in_=draw[:], func=AF.Exp, bias=nbias[:],
            scale=MM_SCALE,
        )
        nc.scalar.dma_start(out=out[:], in_=outsb[:])

    nc.compile()
    return nc


def _make_in_maps(z1: np.ndarray, z2: np.ndarray):
    z = np.concatenate([z1, z2], axis=0)  # [8192, 1024] f32
    zt = (z * FP8_SCALE).astype(np.float32).T  # [D, ROWS]
    # [NP, 128, KT, NTILE]: per 512-row panel, contiguous [kp, k, n] tiles
    zcb = np.ascontiguousarray(
        zt.reshape(KT, 128, NP, NTILE).transpose(2, 1, 0, 3)
    ).astype(ml_dtypes.float8_e4m3)
    in_maps = []
    for c in range(N_CORES):
        order = [(s + c) % NP for s in range(NP)]
        in_maps.append({"z": np.ascontiguousarray(zcb[order])})
    return in_maps


def kernel(z1: np.ndarray, z2: np.ndarray) -> np.ndarray:
    assert z1.shape == (N, D) and z2.shape == (N, D)
    in_maps = _make_in_maps(z1, z2)

    if "nc" not in _CACHE:
        _CACHE["nc"] = _build_graph()
    res = run_bass_kernel_spmd(_CACHE["nc"], in_maps, core_ids=list(range(N_CORES)))

    S = np.zeros((NP, NTILE), dtype=np.float64)
    pos_sum = 0.0
    for c, r in enumerate(res.results):
        o = np.asarray(r["out"], dtype=np.float64)      # [128, 20]
        cols = np.asarray(r["cols"], dtype=np.float64)  # [15, 128, 512]
        p0, p8 = c, (8 + c) % NP
        for q in range(4):
            sl = slice(128 * q, 128 * q + 128)
            S[p0, sl] += o[:, q] - o[:, 8 + q]
            S[p8, sl] += o[:, 4 + q] - o[:, 12 + q]
        colr = cols.sum(axis=1)
        colr[14] = np.asarray(r["tailex"], dtype=np.float64).sum(axis=(0, 1))                         # [15, 512]
        for v in range(1, NP):
            S[(v + c) % NP] += colr[v - 1]
        pos_sum += o[:, 16:20].sum() * MM_SCALE
    loss = np.log(S).mean() + INV_T - pos_sum / N
    return np.asarray(loss, dtype=np.float32)
